# revision 53
# baseline (speedup 1.0000x reference)
"""Trainium2 Bass kernel for linear attention (elu+1 feature map).

Reference computation (B=4, N=M=8192, C=512, H=8, D=64):
    kv   = ref @ kv_w.T              -> k, v  [B,H,N,D]
    q    = tgt @ q_w.T               -> [B,H,M,D];  q,k -> elu(x)+1
    ctx  = sum_n k v^T per head      -> [B,H,D,D];  ksum = sum_n k
    x    = (q @ ctx) * SCALE / (1e-6 + q . ksum)
    out  = x @ proj_w.T + proj_b     -> [B,M,C]

Sharding: 8 cores = 4 batches x 2 row-halves. Each core computes partial
ctx/ksum from its half of N, pair-AllReduces the tiny per-head state, then
produces its half of M rows of the output.

Engine balance (trace-driven): Scalar(ACT) was the baseline bottleneck, so
elementwise work is spread deliberately:
  phase 1 per tile:  DVE min(pk,0) -> ACT exp -> DVE stt combine; v copy ACT.
  state:             block-diagonal [128,516] bf16 payload (ctx blocks scaled
                     by SCALE at the PSUM->SBUF copy, 4 ksum columns at the
                     end), pair-AllReduced in bf16 after a tiny warmup
                     AllReduce absorbs the CC stream's first-use trigger
                     latency (~11us -> ~1us); the reduced tile is used
                     directly as the x-matmul lhsT, and the KselW blocks are
                     built on the otherwise-idle gpsimd queue.
  phase 2 per chunk: den matmuls use KselW (per-head ksum column replicated
                     across its 64 block columns) so the PE emits the
                     denominator pre-broadcast to every partition; one
                     DVE reciprocal_approx_fast per pair, one DVE multiply;
                     out-proj is feature-major (proj weights stationary,
                     per-partition bias added on ACT Identity), the [C,R]
                     output is transposed back on the host.
"""

import numpy as np
import ml_dtypes

import concourse.bass as bass
import concourse.mybir as mybir
from concourse import bacc
from concourse.tile import TileContext
from concourse.bass import ts
from concourse.bass_utils import run_bass_kernel_spmd

B, N, M, C, H = 4, 8192, 8192, 512, 8
D = C // H
SCALE = D**-0.5
NCORES = 8
BF = mybir.dt.bfloat16
F32 = mybir.dt.float32

_CACHE = {}


def build(R_ref, R_q, num_devices, replica_groups, lookahead=3):
    """Emit the SPMD graph. R_ref/R_q = rows of the ref/target shard."""
    P = 128
    KC = C // P          # 4 c-chunks
    NT1 = R_ref // P     # phase-1 row tiles
    CH = 512             # phase-2 chunk (columns of rows)
    NCH = R_q // CH      # phase-2 chunks
    NPAIR = H // 2       # head pairs
    VW = C + NPAIR       # 516: v tiles, 4 pair groups x (128 v + 1 ones)
    CP = C + NPAIR       # 516: block-diag ctx (512) + 4 ksum columns
    STATE = P * CP       # collective payload elements (bf16)

    nc = bacc.Bacc("TRN2", target_bir_lowering=False, debug=False,
                   num_devices=num_devices)

    refT = nc.dram_tensor("refT", [C, R_ref], BF, kind="ExternalInput")
    tgtT = nc.dram_tensor("tgtT", [C, R_q], BF, kind="ExternalInput")
    kv_wT = nc.dram_tensor("kv_wT", [C, 2 * C], BF, kind="ExternalInput")
    q_wT = nc.dram_tensor("q_wT", [C, C], BF, kind="ExternalInput")
    proj_wT = nc.dram_tensor("proj_wT", [C, C], BF, kind="ExternalInput")
    bias_b = nc.dram_tensor("bias_b", [P, KC], F32, kind="ExternalInput")
    out_ext = nc.dram_tensor("out", [C, R_q], BF, kind="ExternalOutput")
    cc_in = nc.dram_tensor("cc_in", [STATE], BF)
    cc_out = nc.dram_tensor("cc_out", [STATE], BF)
    cc_w_in = nc.dram_tensor("cc_w_in", [P], BF)
    cc_w_out = nc.dram_tensor("cc_w_out", [P], BF)

    with TileContext(nc) as tc:
        with (
            tc.tile_pool(name="res", bufs=1) as res,
            tc.tile_pool(name="kv", bufs=4) as kvp,
            tc.tile_pool(name="tmp", bufs=4) as tmp,
            tc.tile_pool(name="rc", bufs=3) as rcp,
            tc.tile_pool(name="qte", bufs=1) as qtep,
            tc.tile_pool(name="xt", bufs=2 + lookahead) as xtp,
            tc.tile_pool(name="o", bufs=6) as op_,
        ):
            # ---- resident inputs ----
            # kv weights (k-half first) + refT pieces first so phase 1 can
            # start after ~1MB of DMA instead of the full 18MB.
            NPIECE = 8
            PC_R = R_ref // NPIECE
            PC_Q = R_q // NPIECE
            kvw_sb = []
            for kc in range(KC):
                t = res.tile([P, 2 * C], BF, tag=f"kvw{kc}")
                nc.sync.dma_start(t[:, 0:C], kv_wT[ts(kc, P), 0:C])
                kvw_sb.append(t)
            refT_sb = [res.tile([P, R_ref], BF, tag=f"refT{kc}",
                                name=f"refT_sb{kc}") for kc in range(KC)]
            for kc in range(KC):
                nc.sync.dma_start(refT_sb[kc][:, ts(0, PC_R)],
                                  refT[ts(kc, P), ts(0, PC_R)])
            # warm up the CC stream during phase 1 so the real AllReduce
            # doesn't pay the first-use trigger latency
            wt = res.tile([1, P], BF, tag="ccw")
            nc.vector.memset(wt[:], 0.0)
            nc.sync.dma_start(cc_w_in[:].rearrange("(p f) -> p f", p=1),
                              wt[:])
            nc.gpsimd.collective_compute(
                "AllReduce", mybir.AluOpType.add,
                replica_groups=replica_groups,
                ins=[cc_w_in[:]], outs=[cc_w_out[:]])
            for kc in range(KC):
                nc.sync.dma_start(kvw_sb[kc][:, C : 2 * C],
                                  kv_wT[ts(kc, P), C : 2 * C])
            for pc in range(1, NPIECE):
                for kc in range(KC):
                    nc.sync.dma_start(refT_sb[kc][:, ts(pc, PC_R)],
                                      refT[ts(kc, P), ts(pc, PC_R)])
            qw_sb = []
            pw_sb = []
            for kc in range(KC):
                t = res.tile([P, C], BF, tag=f"qw{kc}")
                nc.sync.dma_start(t[:], q_wT[ts(kc, P), :])
                qw_sb.append(t)
                t = res.tile([P, C], BF, tag=f"pw{kc}")
                nc.sync.dma_start(t[:], proj_wT[ts(kc, P), :])
                pw_sb.append(t)
            tgtT_sb = [res.tile([P, R_q], BF, tag=f"tgtT{kc}",
                                name=f"tgtT_sb{kc}") for kc in range(KC)]
            for pc in range(NPIECE):
                for kc in range(KC):
                    nc.sync.dma_start(tgtT_sb[kc][:, ts(pc, PC_Q)],
                                      tgtT[ts(kc, P), ts(pc, PC_Q)])
            bias_sb = res.tile([P, KC], F32, tag="bias")
            nc.sync.dma_start(bias_sb[:], bias_b[:, :])

            # zero-init hoisted here: no dependency, keeps the
            # post-collective critical path to just the tiny KselW builds
            ctx_cp = res.tile([P, CP], BF, tag="ctx_cp")
            nc.vector.memset(ctx_cp[:], 0.0)
            # KselW[p]: block-diag [128,128] with each 64x64 block holding
            # the head's ksum column replicated across the block columns --
            # the den matmul then emits the denominator already replicated
            # to every partition of the pair tile (no broadcast needed).
            KselW = []
            for p in range(NPAIR):
                s = res.tile([P, P], BF, tag=f"ksw{p}", name=f"KselW{p}")
                nc.vector.memset(s[:], 0.0)
                KselW.append(s)

            # ---- phase 1: kv, elu(k), ctx+ksum ----
            # v tiles are resident with a constant ones column per pair, so
            # each pair's ctx matmul also accumulates ksum (col 128); the
            # diagonal 64x64 blocks hold the two heads' ctx, off-diagonal
            # blocks are ignored garbage.
            VN = 3
            v_res = [res.tile([P, VW], BF, tag=f"vres{r}", name=f"v_res{r}")
                     for r in range(VN)]
            for r in range(VN):
                ones_view = v_res[r][:].rearrange(
                    "p (g c) -> p g c", c=P + 1)[:, :, P : P + 1]
                nc.vector.memset(ones_view, 1.0)

            qte = [[None] * KC for _ in range(NCH)]

            def qt_chunk(j):
                for mc in range(KC):
                    pq = p2ps.tile([P, CH], F32, tag="mm", bufs=3)
                    for kc in range(KC):
                        nc.tensor.matmul(pq[:], qw_sb[kc][:, ts(mc, P)],
                                         tgtT_sb[kc][:, ts(j, CH)],
                                         start=(kc == 0), stop=(kc == KC - 1))
                    mn = tmp.tile([P, CH], BF, tag="mn")
                    nc.scalar.activation(mn[:], pq[:],
                                         mybir.ActivationFunctionType.Relu,
                                         scale=-1.0)
                    ex = tmp.tile([P, CH], BF, tag="ex")
                    nc.scalar.activation(ex[:], mn[:],
                                         mybir.ActivationFunctionType.Exp,
                                         scale=-1.0)
                    q_sb = qtep.tile([P, CH], BF, tag=f"qte{j}_{mc}",
                                     name=f"qte{j}_{mc}")
                    nc.vector.scalar_tensor_tensor(
                        q_sb[:], pq[:], 0.0, ex[:],
                        mybir.AluOpType.max, mybir.AluOpType.add)
                    qte[j][mc] = q_sb

            pacc = tc.alloc_tile_pool(name="acc", bufs=1, space="PSUM")
            ctx_ps = [pacc.tile([P, P + 1], F32, tag=f"ctx{p}",
                                name=f"ctx_ps{p}") for p in range(NPAIR)]
            p1ps = tc.alloc_tile_pool(name="p1ps", bufs=1, space="PSUM")

            def ctx_mms(ip, k_sb, v_sb):
                # ctx+ksum accumulate per head pair (one matmul each)
                for p in range(NPAIR):
                    nc.tensor.matmul(
                        ctx_ps[p][:], k_sb[:, ts(p, P)],
                        v_sb[:, p * (P + 1) : (p + 1) * (P + 1)],
                        start=(ip == 0), stop=(ip == NT1 - 1))

            prev = None
            for i in range(NT1):
                pk = p1ps.tile([P, C], F32, tag="pk", bufs=2, name="pk")
                pv = p1ps.tile([P, C], F32, tag="pv", bufs=2, name="pv")
                for kc in range(KC):
                    lhsT = refT_sb[kc][:, ts(i, P)]
                    nc.tensor.matmul(pk[:], lhsT, kvw_sb[kc][:, 0:C],
                                     start=(kc == 0), stop=(kc == KC - 1))
                    nc.tensor.matmul(pv[:], lhsT, kvw_sb[kc][:, C : 2 * C],
                                     start=(kc == 0), stop=(kc == KC - 1))
                # tile i-1's ctx matmuls are emitted AFTER tile i's
                # projections: the PE keeps streaming while tile i's elu
                # chain (DVE min -> ACT exp -> DVE combine) is in flight.
                if prev is not None:
                    ctx_mms(*prev)
                # elu(x)+1 = max(x,0) + exp(min(x, 0)); min on DVE, exp on
                # ACT, combine on DVE -- splits the load ACT:1 DVE:2 (ACT
                # also carries the v copy, DVE is idle here otherwise)
                mn = tmp.tile([P, C], BF, tag="mn")
                nc.vector.tensor_scalar_min(mn[:], pk[:], 0.0)
                ex = tmp.tile([P, C], BF, tag="ex")
                nc.scalar.activation(ex[:], mn[:],
                                     mybir.ActivationFunctionType.Exp)
                k_sb = kvp.tile([P, C], BF, tag="k")
                nc.vector.scalar_tensor_tensor(
                    k_sb[:], pk[:], 0.0, ex[:],
                    mybir.AluOpType.max, mybir.AluOpType.add)
                v_sb = v_res[i % VN]
                v_view = v_sb[:].rearrange("p (g c) -> p g c",
                                           c=P + 1)[:, :, 0:P]
                nc.scalar.activation(
                    v_view, pv[:].rearrange("p (g c) -> p g c", c=P),
                    mybir.ActivationFunctionType.Copy)
                prev = (i, k_sb, v_sb)
            ctx_mms(*prev)

            # ---- collective: pair AllReduce of ctx + ksum (bf16) ----
            # The AR is latency-bound (~9-14us) and insensitive to payload
            # size, so ship the block-diagonal layout directly: the reduced
            # tile IS the x-matmul lhsT with no post-collective unpack.
            # ctx_cp is pre-zeroed; only the valid diagonal 64x64 blocks
            # (scaled by SCALE so phase 2 never multiplies by it) and the
            # per-pair ksum columns (512..515) are copied. The ctx blocks
            # go on ACT, the ksum columns on DVE, so the copies drain both
            # queues in parallel right at the phase-1 tail.
            for p in range(NPAIR):
                nc.scalar.activation(
                    ctx_cp[0:D, p * P : p * P + D], ctx_ps[p][0:D, 0:D],
                    mybir.ActivationFunctionType.Copy, scale=SCALE)
                nc.scalar.activation(
                    ctx_cp[D:P, p * P + D : (p + 1) * P],
                    ctx_ps[p][D:P, D:P],
                    mybir.ActivationFunctionType.Copy, scale=SCALE)
                nc.vector.tensor_copy(ctx_cp[:, C + p : C + p + 1],
                                      ctx_ps[p][:, P : P + 1])
            p1ps.release()
            pacc.release()
            nc.sync.dma_start(
                cc_in[:].rearrange("(p f) -> p f", p=P), ctx_cp[:])
            nc.gpsimd.collective_compute(
                "AllReduce", mybir.AluOpType.add,
                replica_groups=replica_groups,
                ins=[cc_in[:]], outs=[cc_out[:]])

            # collective results: ctxr[:, 0:512] is used directly as the
            # block-diagonal x lhsT; KselW blocks are built after the qt
            # chunks are emitted (see below) so the ACT queue doesn't block
            # on the collective.
            ctxr = res.tile([P, CP], BF, tag="ctxr", name="ctxr")
            nc.sync.dma_start(
                ctxr[:], cc_out[:].rearrange("(p f) -> p f", p=P))
            # Post-collective state builds go on the gpsimd queue: it is
            # idle, and its FIFO is already ordered behind the collective
            # -- putting these on ACT/DVE would head-of-line-block the qt
            # elu stream until the collective lands. tensor_scalar_add
            # with a per-partition scalar (fp32 staging copy of the ksum
            # columns) replicates each head's ksum column across its 64
            # block columns (in0 is the pre-zeroed block).
            ksum_f = res.tile([P, NPAIR], F32, tag="ksumf")
            nc.gpsimd.tensor_copy(ksum_f[:], ctxr[:, C:CP])
            for p in range(NPAIR):
                nc.gpsimd.tensor_scalar_add(
                    KselW[p][0:D, 0:D], KselW[p][0:D, 0:D],
                    ksum_f[0:D, p : p + 1])
                nc.gpsimd.tensor_scalar_add(
                    KselW[p][D:P, D:P], KselW[p][D:P, D:P],
                    ksum_f[D:P, p : p + 1])

            # ---- phase 2b: A(j) = denom/recip/x per chunk, B(j) = out-proj;
            # emitted with `lookahead` A-stages ahead of each B-stage so the
            # PE stream always has independent matmuls while DVE/ACT finish
            # the previous chunks.
            p2ps = tc.alloc_tile_pool(name="p2ps", bufs=1, space="PSUM")

            def stage_a(j):
                # den_wide = KselW^T @ q' comes out of the PE with the
                # per-head denominator already on every partition of the
                # pair block; one fused-Newton reciprocal per pair, then
                # the multiply. (The reference's +1e-6 guard is dropped:
                # den ~ 1e3..1e6 here, it is far below bf16 resolution.)
                xts = []
                for p in range(NPAIR):
                    # px first: it only needs the AllReduced tile, while
                    # den also waits on the gpsimd KselW build
                    px = p2ps.tile([P, CH], F32, tag="px", bufs=3,
                                   name="px")
                    nc.tensor.matmul(px[:], ctxr[:, ts(p, P)],
                                     qte[j][p][:], start=True, stop=True)
                    dw = p2ps.tile([P, CH], F32, tag="den", bufs=2,
                                   name="den")
                    nc.tensor.matmul(dw[:], KselW[p][:], qte[j][p][:],
                                     start=True, stop=True)
                    rw = rcp.tile([P, CH], F32, tag="rec")
                    nc.vector.reciprocal_approx_fast(rw[:], dw[:])
                    xt = xtp.tile([P, CH], BF, tag=f"xt{p}")
                    nc.vector.tensor_mul(xt[:], px[:], rw[:])
                    xts.append(xt)
                return xts

            def stage_b(j, xts):
                # feature-major out-proj: out^T[co,m] accumulated with the
                # proj weights as the stationary operand; per-partition bias
                # is added during the PSUM->SBUF move, alternating engines.
                for co in range(KC):
                    po = p2ps.tile([P, CH], F32, tag="mm", bufs=3,
                                   name="po")
                    for kc in range(KC):
                        nc.tensor.matmul(po[:], pw_sb[kc][:, ts(co, P)],
                                         xts[kc][:], start=(kc == 0),
                                         stop=(kc == KC - 1))
                    o_sb = op_.tile([P, CH], BF, tag="o")
                    nc.scalar.activation(
                        o_sb[:], po[:],
                        mybir.ActivationFunctionType.Identity,
                        bias=bias_sb[:, co : co + 1])
                    nc.sync.dma_start(out_ext[ts(co, P), ts(j, CH)], o_sb[:])

            # Interleave: qt chunks lead the stage pipeline by QT_LEAD so
            # the PE FIFO always holds independent matmuls (den/px/out-proj
            # of earlier chunks) during the ACT-paced qt PSUM recycling,
            # and stage_b lags stage_a by `lookahead`.
            QT_LEAD = 3
            for j in range(min(QT_LEAD, NCH)):
                qt_chunk(j)
            pend = []
            for j in range(NCH):
                if j + QT_LEAD < NCH:
                    qt_chunk(j + QT_LEAD)
                pend.append((j, stage_a(j)))
                if len(pend) > lookahead:
                    jj, xx = pend.pop(0)
                    stage_b(jj, xx)
            for jj, xx in pend:
                stage_b(jj, xx)
            p2ps.release()
    nc.compile()
    return nc


def _shard_inputs(target_data, reference_data, q_w, kv_w, proj_w, proj_b,
                  R, ncores):
    bf = ml_dtypes.bfloat16
    kv_wT = np.ascontiguousarray(kv_w.T).astype(bf)
    q_wT = np.ascontiguousarray(q_w.T).astype(bf)
    proj_wT = np.ascontiguousarray(proj_w.T).astype(bf)
    bias_b = np.ascontiguousarray(
        np.asarray(proj_b, dtype=np.float32).reshape(C // 128, 128).T)
    in_maps = []
    for c in range(ncores):
        b, half = divmod(c, 2)
        sl = slice(half * R, (half + 1) * R)
        in_maps.append({
            "refT": np.ascontiguousarray(
                np.asarray(reference_data)[b, sl, :].T).astype(bf),
            "tgtT": np.ascontiguousarray(
                np.asarray(target_data)[b, sl, :].T).astype(bf),
            "kv_wT": kv_wT, "q_wT": q_wT, "proj_wT": proj_wT,
            "bias_b": bias_b,
        })
    return in_maps


def kernel(target_data, reference_data, q_w, kv_w, proj_w, proj_b):
    R = M // 2
    key = (R, NCORES)
    if key not in _CACHE:
        _CACHE[key] = build(R, R, NCORES,
                            [[0, 1], [2, 3], [4, 5], [6, 7]], lookahead=1)
    nc = _CACHE[key]
    in_maps = _shard_inputs(target_data, reference_data, q_w, kv_w, proj_w,
                            proj_b, R, NCORES)
    res = run_bass_kernel_spmd(nc, in_maps, list(range(NCORES)))
    out = np.empty((B, M, C), dtype=np.float32)
    for c in range(NCORES):
        b, half = divmod(c, 2)
        out[b, half * R : (half + 1) * R, :] = np.asarray(
            res.results[c]["out"], dtype=np.float32).T
    return out
